# revision 1
# baseline (speedup 1.0000x reference)
"""Trainium2 Bass kernel v6: batched inverse of homogeneous affine transforms.

v5 (host SoA transpose, all-flat DVE ops) + software-pipelined emission:
next chunk's ACT prep (ytile/tneg) is emitted before this chunk's rdet
replication so the in-order ACT queue never gates the DVE at chunk
boundaries; first/last chunks are smaller to cut pipeline fill/drain.
"""

import numpy as np

B = 4_194_304
NCORES = 8
BL = B // NCORES
P = 128
CHUNKS = [64, 192, 256] + [512] * 7  # ramp-up; sums to BL/P = 4096
assert sum(CHUNKS) == BL // P


def _V(base_ap, off, dims):
    import concourse.bass as bass

    return bass.AP(
        base_ap.tensor,
        base_ap.offset + off,
        [list(base_ap.ap[0])] + [[int(s), int(n)] for s, n in dims],
    )


def build_nc(bl=BL, chunks=None):
    import concourse.bass as bass
    import concourse.bacc as bacc
    import concourse.mybir as mybir
    from concourse.tile import TileContext

    chunks = chunks or CHUNKS
    assert sum(chunks) * P == bl
    f32 = mybir.dt.float32

    nc = bacc.Bacc()
    trf = nc.declare_dram_parameter("trf", [12, bl], f32, isOutput=False)
    out = nc.declare_dram_parameter("out", [12, bl], f32, isOutput=True)
    V = nc.vector

    bases = []
    acc = 0
    for c in chunks:
        bases.append(acc)
        acc += P * c

    def dram_ap(t, base, c):
        return bass.AP(t.ap().tensor, base, [[c, P], [bl, 12], [1, c]])

    with TileContext(nc) as tc:
        with (
            tc.tile_pool(name="io", bufs=2) as io,
            tc.tile_pool(name="tmp", bufs=1) as tmp,
            tc.tile_pool(name="tng", bufs=2) as tng,
            tc.tile_pool(name="pqp", bufs=2) as pqp,
        ):

            def prep(n):
                c = chunks[n]
                tin = io.tile([P, 12 * c], f32, tag="tin")
                nc.sync.dma_start(out=tin[:], in_=dram_ap(trf, bases[n], c))
                return {"tin": tin, "c": c, "n": n}

            def act_prep(st):
                c, tin = st["c"], st["tin"]
                ytile = tmp.tile([P, 9 * c], f32, tag="ytile")
                nc.scalar.copy(_V(ytile, 0, [(1, 6 * c)]),
                               _V(tin, 3 * c, [(1, 6 * c)]))
                nc.scalar.copy(_V(ytile, 6 * c, [(1, 3 * c)]),
                               _V(tin, 0, [(1, 3 * c)]))
                tneg3 = tng.tile([P, 3 * c], f32, tag="tneg3")
                nc.scalar.mul(_V(tneg3, 0, [(1, 3 * c)]),
                              _V(tin, 9 * c, [(1, 3 * c)]), -1.0)
                st["ytile"], st["tneg3"] = ytile, tneg3

            def head(st):
                c, tin, ytile = st["c"], st["tin"], st["ytile"]
                pq = pqp.tile([P, 9 * c], f32, tag="pq")
                tout = io.tile([P, 12 * c], f32, tag="tout")
                for j in range(3):
                    V.tensor_mul(
                        _V(pq, j * c, [(3 * c, 3), (1, c)]),
                        _V(tin, ((j + 1) % 3) * c, [(3 * c, 3), (1, c)]),
                        _V(ytile, ((j + 2) % 3) * c, [(3 * c, 3), (1, c)]),
                    )
                for j in range(3):
                    V.tensor_mul(
                        _V(tout, j * c, [(3 * c, 3), (1, c)]),
                        _V(tin, ((j + 2) % 3) * c, [(3 * c, 3), (1, c)]),
                        _V(ytile, ((j + 1) % 3) * c, [(3 * c, 3), (1, c)]),
                    )
                V.tensor_sub(_V(pq, 0, [(1, 9 * c)]),
                             _V(pq, 0, [(1, 9 * c)]),
                             _V(tout, 0, [(1, 9 * c)]))
                tm = tmp.tile([P, 3 * c], f32, tag="tm")
                V.tensor_mul(_V(tm, 0, [(1, 3 * c)]),
                             _V(tin, 0, [(1, 3 * c)]),
                             _V(pq, 3 * c, [(1, 3 * c)]))
                d1 = tmp.tile([P, c], f32, tag="d1")
                det = tmp.tile([P, c], f32, tag="det")
                V.tensor_add(d1[:], _V(tm, 0, [(1, c)]), _V(tm, c, [(1, c)]))
                V.tensor_add(det[:], d1[:], _V(tm, 2 * c, [(1, c)]))
                rdet3 = tmp.tile([P, 3 * c], f32, tag="rdet3")
                V.reciprocal_approx_fast(_V(rdet3, 0, [(1, c)]), det[:])
                st["pq"], st["tout"], st["rdet3"] = pq, tout, rdet3

            def rdet_rep(st):
                c, rdet3 = st["c"], st["rdet3"]
                nc.scalar.copy(_V(rdet3, c, [(1, c)]), _V(rdet3, 0, [(1, c)]))
                nc.scalar.copy(_V(rdet3, 2 * c, [(1, c)]), _V(rdet3, 0, [(1, c)]))

            def tail(st):
                c, pq, tout = st["c"], st["pq"], st["tout"]
                tneg3, rdet3, n = st["tneg3"], st["rdet3"], st["n"]
                # u[3r+j] = Z[3r+j] * tneg[j]: flat per-r into tout scratch
                for r in range(3):
                    V.tensor_mul(_V(tout, 3 * r * c, [(1, 3 * c)]),
                                 _V(pq, 3 * r * c, [(1, 3 * c)]),
                                 _V(tneg3, 0, [(1, 3 * c)]))
                e1 = tmp.tile([P, 3 * c], f32, tag="e1")
                V.tensor_add(_V(e1, 0, [(c, 3), (1, c)]),
                             _V(tout, 0, [(3 * c, 3), (1, c)]),
                             _V(tout, c, [(3 * c, 3), (1, c)]))
                V.tensor_add(_V(e1, 0, [(c, 3), (1, c)]),
                             _V(e1, 0, [(c, 3), (1, c)]),
                             _V(tout, 2 * c, [(3 * c, 3), (1, c)]))
                # inv = Z * rdet (flat per-r), col3 = e1 * rdet
                for r in range(3):
                    V.tensor_mul(_V(tout, 3 * r * c, [(1, 3 * c)]),
                                 _V(pq, 3 * r * c, [(1, 3 * c)]),
                                 _V(rdet3, 0, [(1, 3 * c)]))
                V.tensor_mul(_V(tout, 9 * c, [(1, 3 * c)]),
                             _V(e1, 0, [(1, 3 * c)]),
                             _V(rdet3, 0, [(1, 3 * c)]))
                nc.sync.dma_start(out=dram_ap(out, bases[n], c), in_=tout[:])

            nch = len(chunks)
            states = [None] * nch
            states[0] = prep(0)
            act_prep(states[0])
            for n in range(nch):
                st = states[n]
                if n + 1 < nch:
                    states[n + 1] = prep(n + 1)
                head(st)
                if n + 1 < nch:
                    act_prep(states[n + 1])
                rdet_rep(st)
                tail(st)

    return nc


_CACHE = {}


def _get_nc():
    if "nc" not in _CACHE:
        nc = build_nc()
        nc.finalize()
        _CACHE["nc"] = nc
    return _CACHE["nc"]


def _prep_inputs(trf):
    x = np.asarray(trf, dtype=np.float32).reshape(B, 3, 4).copy()
    x[:, 0, 0] += 1.0
    x[:, 1, 1] += 1.0
    x[:, 2, 2] += 1.0
    xt = x.reshape(NCORES, BL, 3, 4).transpose(0, 3, 2, 1)[:, [1, 2, 0, 3]]
    return np.ascontiguousarray(xt.reshape(NCORES, 12, BL))


def _decode_outputs(outs):
    inv = outs[:, :9].reshape(NCORES, 3, 3, BL)
    col3 = outs[:, 9:12]
    res = np.empty((NCORES, BL, 3, 4), np.float32)
    res[..., :3] = inv.transpose(0, 3, 1, 2)
    res[..., 3] = col3.transpose(0, 2, 1)
    return res.reshape(B, 3, 4)


def run(trf, trace=False, **spmd_kwargs):
    from concourse.bass_utils import run_bass_kernel_spmd

    xin = _prep_inputs(trf)
    in_maps = [{"trf": xin[i]} for i in range(NCORES)]
    nc = _get_nc()
    res = run_bass_kernel_spmd(
        nc, in_maps, list(range(NCORES)), trace=trace, **spmd_kwargs
    )
    outs = np.stack([np.asarray(res.results[i]["out"]) for i in range(NCORES)])
    return _decode_outputs(outs), res


def kernel(trf):
    return run(trf)[0]

